# revision 22
# baseline (speedup 1.0000x reference)
"""Distributed GCN (2-layer + readout) on 8 Trainium2 NeuronCores.

Src-sharded gather + one-hot-matmul aggregation + bf16 ReduceScatter:

Nodes are sharded 8-way by SRC owner (contiguous blocks of NSH=12544,
tile-aligned). Each GCN layer's linear W is folded to AFTER the
collective (A @ (hW) == (A @ h) @ W), so the gather table is just
u = dinv * h — elementwise, no matmul on the critical path before
gathering. Tables are stored as 256B rows (64 bf16 payload + 64 bf16
junk) to satisfy dma_gather's 256B-multiple elem constraint; gathers
are purely core-local (no AllGather).

Messages (edges) are sorted by dst and packed into 128-message columns
within tile-PAIRS (2x128 dst nodes); per pair the column count is the
max over cores so the instruction schedule is core-uniform. Each column
is aggregated with 1-2 TensorE matmuls: lhsT = a one-hot selection
matrix S (built on DVE via is_equal against an iota row; bf16) and
rhs = the gathered messages; PSUM accumulates per dst tile across a
bank of 8 tiles, then is copied (cast bf16) and spilled to a DRAM
accumulator. One bf16 ReduceScatter per layer replaces the baseline's
two f32 AllGathers; the received shard gets self-loop + @W + epilogue
via PE transposes (engines otherwise idle). This removes the combine
gather stage (~100k descriptors/layer) and the AllGather-before-gather
barrier that dominated the scatter/gather baseline.
"""
import numpy as np
import ml_dtypes

from concourse import bass, bacc, tile, mybir, bass_utils

F32 = mybir.dt.float32
BF16 = mybir.dt.bfloat16
I16 = mybir.dt.int16
I32 = mybir.dt.int32
NPBF16 = ml_dtypes.bfloat16

NCORES = 8
D = 64
CB = 32          # columns per gather block


def _roundup(x, m):
    return (x + m - 1) // m * m


def preprocess(edge_index, n_nodes):
    src = np.asarray(edge_index[0], dtype=np.int64)
    dst = np.asarray(edge_index[1], dtype=np.int64)
    E = len(src)

    NSH = _roundup((n_nodes + NCORES - 1) // NCORES, 256)  # shard size
    assert NSH * NCORES >= n_nodes and NSH < 32768
    NTOWN = NSH // 128            # own tiles per core (even)
    NT = NCORES * NTOWN           # global dst tiles
    NPAIR = NT // 2
    assert NPAIR % 8 == 0

    owner = src // NSH
    srow = src % NSH
    pair = dst // 256
    rel = dst - pair * 256        # [0, 256)

    deg = np.bincount(dst, minlength=NSH * NCORES).astype(np.float32) + 1.0
    deg_tiles = []
    for c in range(NCORES):
        d = np.ones(NSH, np.float32)
        lo = c * NSH
        d[:] = deg[lo:lo + NSH]
        d[max(0, n_nodes - lo):] = 1.0  # pad nodes
        deg_tiles.append(np.ascontiguousarray(d.reshape(NTOWN, 128).T))

    # per (core, pair) message lists, sorted by dst
    order = np.lexsort((dst, pair, owner))
    so, sp, sr, sl = owner[order], pair[order], srow[order], rel[order]
    # counts per core,pair
    cnt = np.zeros((NCORES, NPAIR), np.int64)
    np.add.at(cnt, (so, sp), 1)
    ncols = np.maximum(1, (cnt.max(axis=0) + 127) // 128)  # per pair
    col_base = np.concatenate([[0], np.cumsum(ncols)])
    NCOL = int(col_base[-1])
    NPOS = NCOL * 128

    # fill per-core vals (gather idx) and rel arrays
    vals = np.zeros((NCORES, NPOS), np.int16)
    rlo = np.full((NCORES, NPOS), -1.0, np.float32)
    # start offsets of each (core,pair) run inside the sorted arrays
    run_start = np.zeros((NCORES, NPAIR + 1), np.int64)
    csum = np.cumsum(cnt, axis=1)
    run_start[:, 1:] = csum
    core_off = np.concatenate([[0], np.cumsum(csum[:, -1])])[:-1]
    for c in range(NCORES):
        for k in range(NPAIR):
            a = core_off[c] + run_start[c, k]
            b = core_off[c] + run_start[c, k + 1]
            m = b - a
            if m == 0:
                continue
            q = np.arange(m) + col_base[k] * 128
            vals[c, q] = sr[a:b]
            rlo[c, q] = sl[a:b]

    # crossing flags per column (shared across cores)
    rel_by_col = rlo.reshape(NCORES, NCOL, 128)
    crossing = (rel_by_col >= 128.0).any(axis=(0, 2))
    # force a crossing col on pairs whose odd tile would otherwise never
    # be written (pair empty or no rel>=128 anywhere)
    for k in range(NPAIR):
        c0, c1 = col_base[k], col_base[k + 1]
        if not crossing[c0:c1].any():
            crossing[c1 - 1] = True
    xid_of = np.full(NCOL, -1, np.int64)
    xid_of[crossing] = np.arange(int(crossing.sum()))
    NX = int(crossing.sum())

    # compact hi-rel arrays
    rhi = np.full((NCORES, NX, 128), -1.0, np.float32)
    for c in range(NCORES):
        rb = rel_by_col[c][crossing]  # [NX, 128]
        hi = rb >= 128.0
        rhi[c][hi] = rb[hi] - 128.0

    # pack gather idx: [128, NPOS//16] int16, 16-partition wrap replicated
    gidx_all, rlo_all, rhi_all = [], [], []
    for c in range(NCORES):
        gidx_all.append(np.ascontiguousarray(
            np.tile(vals[c].reshape(-1, 16).T, (8, 1))))
        rlo_all.append(np.ascontiguousarray(
            rel_by_col[c].T).astype(NPBF16))     # [128, NCOL]
        if NX:
            rhi_all.append(np.ascontiguousarray(
                rhi[c].T).astype(NPBF16))        # [128, NX]
        else:
            rhi_all.append(np.zeros((128, 1), NPBF16))

    # column schedule (shared): for each column: pair k, first/last col
    # flags, xid (or -1). The pair's matmuls form ONE psum accumulation
    # group (2KB zero region = bank): start on first col's even matmul,
    # stop on the last emitted matmul of the pair.
    cols = []
    for k in range(NPAIR):
        c0, c1 = col_base[k], col_base[k + 1]
        for c in range(c0, c1):
            last = c == c1 - 1
            xid = int(xid_of[c]) if crossing[c] else -1
            cols.append(dict(
                k=k, first=(c == c0), last=last, xid=xid,
                stop_even=(last and xid < 0),
                stop_odd=(last and xid >= 0),
            ))

    meta = dict(NSH=NSH, NTOWN=NTOWN, NT=NT, NPAIR=NPAIR, NCOL=NCOL,
                NPOS=NPOS, NX=max(NX, 1), cols=cols, n_nodes=n_nodes)
    return meta, gidx_all, rlo_all, rhi_all, deg_tiles


def build(meta):
    NSH, NTOWN, NT = meta["NSH"], meta["NTOWN"], meta["NT"]
    NCOL, NPOS, NX = meta["NCOL"], meta["NPOS"], meta["NX"]
    NPAIR = meta["NPAIR"]
    cols = meta["cols"]

    nc = bacc.Bacc("TRN2", target_bir_lowering=False, debug=False,
                   num_devices=NCORES, num_swdge_queues=4)

    xse = nc.dram_tensor("xs", [128, NTOWN * D], F32, kind="ExternalInput")
    dege = nc.dram_tensor("deg", [128, NTOWN], F32, kind="ExternalInput")
    gidxe = nc.dram_tensor("gidx", [128, NPOS // 16], I16,
                           kind="ExternalInput")
    rloe = nc.dram_tensor("rlo", [128, NCOL], BF16, kind="ExternalInput")
    rhie = nc.dram_tensor("rhi", [128, NX], BF16, kind="ExternalInput")
    W1e = nc.dram_tensor("W1", [D, D], BF16, kind="ExternalInput")
    W2e = nc.dram_tensor("W2", [D, D], BF16, kind="ExternalInput")
    b1e = nc.dram_tensor("b1bc", [128, D], F32, kind="ExternalInput")
    b2e = nc.dram_tensor("b2bc", [128, D], F32, kind="ExternalInput")
    woute = nc.dram_tensor("woutbc", [128, D], F32, kind="ExternalInput")
    boute = nc.dram_tensor("boutbc", [128, 1], F32, kind="ExternalInput")
    idente = nc.dram_tensor("identbf", [128, 128], BF16,
                            kind="ExternalInput")
    iotaxe = nc.dram_tensor("iotax", [128, 128 * CB], BF16,
                            kind="ExternalInput")
    oute = nc.dram_tensor("out", [128, NTOWN], F32, kind="ExternalOutput")

    T0 = nc.dram_tensor("T0", [NSH, 128], BF16)
    T1 = nc.dram_tensor("T1", [NSH, 128], BF16)
    # pair-major accumulator: row = k*256 + p*2 + e  (pair, lane, tile)
    acc = nc.dram_tensor("acc", [NPAIR * 256, D], BF16)
    shard = [nc.dram_tensor(f"shard{L}", [NSH, D], BF16)
             for L in (0, 1)]

    def nodemaj(dram, g):
        # DRAM [(g p), d] viewed as [128, g, d]
        return dram.ap().rearrange("(g p) d -> p g d", p=128)

    def pairmaj(apx):
        # DRAM [(k p e), d] viewed as [128, k, e, d]
        return apx.rearrange("(k p e) d -> p k e d", p=128, e=2)

    with tile.TileContext(nc) as tc:
        with (
            tc.tile_pool(name="const", bufs=1) as pool,
            tc.tile_pool(name="msg", bufs=3) as msgpool,
            tc.tile_pool(name="slo", bufs=2) as slopool,
            tc.tile_pool(name="shi", bufs=2) as shipool,
            tc.tile_pool(name="stage", bufs=4) as stagepool,
            tc.tile_pool(name="aggT", bufs=2) as aggTpool,
            tc.tile_pool(name="hg", bufs=2) as hgpool,
            tc.tile_pool(name="aggps", bufs=4, space="PSUM") as aggps,
            tc.tile_pool(name="trps", bufs=2, space="PSUM") as trps,
            tc.tile_pool(name="mmps", bufs=2, space="PSUM") as mmps,
        ):
            # ---- constants ----
            gidx_t = pool.tile([128, NPOS // 16], I16, tag="gidx")
            rlo_t = pool.tile([128, NCOL], BF16, tag="rlo")
            rhi_t = pool.tile([128, NX], BF16, tag="rhi")
            nc.scalar.dma_start(out=gidx_t[:], in_=gidxe[:])
            nc.scalar.dma_start(out=rlo_t[:], in_=rloe[:])
            nc.scalar.dma_start(out=rhi_t[:], in_=rhie[:])
            W1_t = pool.tile([D, D], BF16, tag="w1")
            W2_t = pool.tile([D, D], BF16, tag="w2")
            b1_t = pool.tile([128, D], F32, tag="b1")
            b2_t = pool.tile([128, D], F32, tag="b2")
            wout_t = pool.tile([128, D], F32, tag="wout")
            bout_t = pool.tile([128, 1], F32, tag="bout")
            ident_t = pool.tile([128, 128], BF16, tag="ident")
            nc.scalar.dma_start(out=W1_t[:], in_=W1e[:])
            nc.scalar.dma_start(out=W2_t[:], in_=W2e[:])
            nc.scalar.dma_start(out=b1_t[:], in_=b1e[:])
            nc.scalar.dma_start(out=b2_t[:], in_=b2e[:])
            nc.scalar.dma_start(out=wout_t[:], in_=woute[:])
            nc.scalar.dma_start(out=bout_t[:], in_=boute[:])
            nc.scalar.dma_start(out=ident_t[:], in_=idente[:])
            deg_t = pool.tile([128, NTOWN], F32, tag="deg")
            nc.sync.dma_start(out=deg_t[:], in_=dege[:])
            dinv_t = pool.tile([128, NTOWN], F32, tag="dinv")
            nc.scalar.activation(dinv_t[:], deg_t[:],
                                 mybir.ActivationFunctionType.Sqrt)
            nc.vector.reciprocal(dinv_t[:], dinv_t[:])
            # iota_x[p, j, c] = j  (host-precomputed; inner strides all 1
            # so the is_equal S-build hits the DVE 2x bf16 fast path)
            iota_x = pool.tile([128, 128, CB], BF16, tag="iotax")
            nc.scalar.dma_start(
                out=iota_x[:],
                in_=iotaxe.ap().rearrange("p (j c) -> p j c", c=CB))

            xs_t = pool.tile([128, NTOWN, D], F32, tag="xs")
            nc.sync.dma_start(
                out=xs_t[:],
                in_=xse.ap().rearrange("p (g d) -> p g d", d=D))

            # ---- u0 = dinv * x into junk-row staging ----
            tjunk = pool.tile([128, NTOWN, 128], BF16, tag="tjunk")
            nc.vector.memset(tjunk[:], 0.0)
            dvb = dinv_t[:].unsqueeze(2).broadcast_to([128, NTOWN, D])
            nc.vector.tensor_tensor(tjunk[:, :, 0:D], xs_t[:], dvb,
                                    mybir.AluOpType.mult)
            nc.sync.dma_start(out=nodemaj(T0, NTOWN), in_=tjunk[:])

            NBLK = (NCOL + CB - 1) // CB
            dma_sems = [nc.alloc_semaphore(f"gdma{q}") for q in range(4)]
            qcnt = [0, 0, 0, 0]

            def agg_layer(L):
                Tbl = T0 if L == 0 else T1
                psum_live = {}
                for blk in range(NBLK):
                    c0 = blk * CB
                    bc = min(CB, NCOL - c0)
                    qn = blk % 4
                    mt = msgpool.tile([128, CB, 128], BF16, tag="m")
                    nc.gpsimd.dma_gather(
                        mt[:, :bc, :], Tbl[:],
                        gidx_t[:, c0 * 8:(c0 + bc) * 8],
                        num_idxs=bc * 128, num_idxs_reg=bc * 128,
                        elem_size=128, single_packet=False,
                        queue_num=qn, prepare_only=True, sem=dma_sems[qn])
                    nc.gpsimd.trigger_dma(count=None, queue_num=qn)
                    qcnt[qn] += 1
                    gtgt = 16 * qcnt[qn]
                    # S_T[p, j, c] = (rel[p, c] == j), built transposed so
                    # every operand's inner stride is 1 (DVE 2x path).
                    # The S builds carry the gather-drain wait; matmuls
                    # (which read mt) depend on S via tracked sems, so the
                    # drain is transitively synchronized.
                    sl = slopool.tile([128, 128, CB], BF16, tag="sl")
                    nc.vector.tensor_tensor(
                        sl[:, :, :bc], iota_x[:, :, :bc],
                        rlo_t[:, c0:c0 + bc].unsqueeze(1).broadcast_to(
                            [128, 128, bc]),
                        mybir.AluOpType.is_equal)._wait_ge(
                            dma_sems[qn], gtgt)
                    # crossing cols in this block -> compact hi one-hots
                    xids = [cols[c]["xid"] for c in range(c0, c0 + bc)
                            if cols[c]["xid"] >= 0]
                    if xids:
                        x0, nxb = xids[0], len(xids)
                        assert xids == list(range(x0, x0 + nxb))
                        sh = shipool.tile([128, 128, CB], BF16, tag="sh")
                        nc.vector.tensor_tensor(
                            sh[:, :, :nxb], iota_x[:, :, :nxb],
                            rhi_t[:, x0:x0 + nxb].unsqueeze(1).broadcast_to(
                                [128, 128, nxb]),
                            mybir.AluOpType.is_equal)._wait_ge(
                                dma_sems[qn], gtgt)
                    for c in range(c0, c0 + bc):
                        info = cols[c]
                        k = info["k"]
                        bank = k // 4
                        slot = k % 4
                        bank_last = slot == 3 or k == NPAIR - 1
                        if bank not in psum_live:
                            psum_live[bank] = aggps.tile(
                                [128, 8, 64], F32, tag="agg", name="aggb")
                        pt = psum_live[bank]
                        j = c - c0
                        nc.tensor.matmul(
                            pt[:, 2 * slot, :], sl[:, :, j], mt[:, j, 0:D],
                            start=info["first"] and slot == 0,
                            stop=info["stop_even"] and bank_last)
                        if info["xid"] >= 0:
                            xj = info["xid"] - x0
                            nc.tensor.matmul(
                                pt[:, 2 * slot + 1, :], sh[:, :, xj],
                                mt[:, j, 0:D],
                                start=False,
                                stop=info["stop_odd"] and bank_last)
                        if info["last"] and bank_last:
                            st = stagepool.tile([128, 8, 64], BF16,
                                                tag="st")
                            nc.vector.tensor_copy(st[:], pt[:])
                            k0 = bank * 4
                            nc.sync.dma_start(
                                out=pairmaj(acc[k0 * 256:(k0 + 4) * 256, :]),
                                in_=st[:].rearrange("p (k e) d -> p k e d",
                                                    e=2))
                            del psum_live[bank]
                assert not psum_live, psum_live.keys()

            def post_layer(L):
                """RS result -> agg_u -> @W -> epilogue."""
                sh_t = pool.tile([128, NTOWN, D], BF16, tag="shards")
                nc.sync.dma_start(
                    out=sh_t[:].rearrange("p (k e) d -> p k e d", e=2),
                    in_=pairmaj(shard[L].ap()))
                # add self-loop term u_own (in tjunk payload half)
                nc.vector.tensor_tensor(sh_t[:], sh_t[:],
                                        tjunk[:, :, 0:D],
                                        mybir.AluOpType.add)
                W_t = W1_t if L == 0 else W2_t
                b_t = b1_t if L == 0 else b2_t
                if L == 1:
                    o_t = pool.tile([128, NTOWN], F32, tag="o")
                for tg in range(0, NTOWN, 8):
                    ng = min(8, NTOWN - tg)
                    tp = trps.tile([64, 8, 128], BF16, tag="tr")
                    for t in range(tg, tg + ng):
                        nc.tensor.transpose(tp[:, t - tg, :], sh_t[:, t, :],
                                            ident_t[:])
                    aT = aggTpool.tile([64, 8, 128], BF16, tag="aT")
                    nc.vector.tensor_copy(aT[:, :ng, :], tp[:, :ng, :])
                    mp = mmps.tile([128, 8, 64], F32, tag="mm")
                    for t in range(tg, tg + ng):
                        nc.tensor.matmul(mp[:, t - tg, :], aT[:, t - tg, :],
                                         W_t[:])
                    hg = hgpool.tile([128, 8, 64], F32, tag="hg")
                    dvg = dinv_t[:, tg:tg + ng].unsqueeze(2).broadcast_to(
                        [128, ng, 64])
                    bbg = b_t[:].unsqueeze(1).broadcast_to([128, ng, 64])
                    nc.vector.tensor_tensor(hg[:, :ng, :], mp[:, :ng, :],
                                            dvg, mybir.AluOpType.mult)
                    nc.vector.tensor_tensor(hg[:, :ng, :], hg[:, :ng, :],
                                            bbg, mybir.AluOpType.add)
                    nc.scalar.activation(hg[:, :ng, :], hg[:, :ng, :],
                                         mybir.ActivationFunctionType.Relu)
                    if L == 0:
                        # u1 = dinv * h into table staging payload
                        nc.vector.tensor_tensor(
                            tjunk[:, tg:tg + ng, 0:D], hg[:, :ng, :], dvg,
                            mybir.AluOpType.mult)
                    else:
                        wbg = wout_t[:].unsqueeze(1).broadcast_to(
                            [128, ng, 64])
                        nc.vector.tensor_tensor(hg[:, :ng, :], hg[:, :ng, :],
                                                wbg, mybir.AluOpType.mult)
                        nc.vector.tensor_reduce(
                            o_t[:, tg:tg + ng], hg[:, :ng, :],
                            axis=mybir.AxisListType.X,
                            op=mybir.AluOpType.add)
                if L == 0:
                    nc.sync.dma_start(out=nodemaj(T1, NTOWN), in_=tjunk[:])
                else:
                    nc.vector.tensor_scalar_add(o_t[:], o_t[:], bout_t[:])
                    nc.sync.dma_start(out=oute[:], in_=o_t[:])

            for L in (0, 1):
                agg_layer(L)
                nc.gpsimd.collective_compute(
                    "ReduceScatter", mybir.AluOpType.add,
                    replica_groups=[list(range(NCORES))],
                    ins=[acc.ap().opt()],
                    outs=[shard[L].ap().opt()])
                post_layer(L)

    nc.compile()
    return nc


_CACHE = {}


def kernel(x, edge_index, batch, W1, b1, W2, b2, Wout, bout, _trace=False):
    x = np.asarray(x, np.float32)
    edge_index = np.asarray(edge_index)
    W1 = np.asarray(W1, np.float32)
    W2 = np.asarray(W2, np.float32)
    b1 = np.asarray(b1, np.float32)
    b2 = np.asarray(b2, np.float32)
    Wout = np.asarray(Wout, np.float32)
    bout = np.asarray(bout, np.float32).reshape(-1)
    N = x.shape[0]

    key = (N, edge_index.shape[1])
    if key not in _CACHE:
        meta, gidx_all, rlo_all, rhi_all, deg_tiles = preprocess(
            edge_index, N)
        nc = build(meta)
        _CACHE[key] = (meta, gidx_all, rlo_all, rhi_all, deg_tiles, nc)
    meta, gidx_all, rlo_all, rhi_all, deg_tiles, nc = _CACHE[key]
    NSH, NTOWN = meta["NSH"], meta["NTOWN"]

    identbf = np.eye(128, dtype=np.float32).astype(NPBF16)
    iotax = np.ascontiguousarray(np.broadcast_to(
        np.arange(128, dtype=np.float32)[None, :, None],
        (128, 128, CB)).reshape(128, 128 * CB)).astype(NPBF16)
    b1bc = np.tile(b1[None, :], (128, 1)).astype(np.float32)
    b2bc = np.tile(b2[None, :], (128, 1)).astype(np.float32)
    woutbc = np.tile(Wout.reshape(1, -1), (128, 1)).astype(np.float32)
    boutbc = np.full((128, 1), float(bout[0]), np.float32)
    W1bf = W1.astype(NPBF16)
    W2bf = W2.astype(NPBF16)

    in_maps = []
    for c in range(NCORES):
        xsh = np.zeros((NSH, D), np.float32)
        lo, hi = c * NSH, min((c + 1) * NSH, N)
        if hi > lo:
            xsh[:hi - lo] = x[lo:hi]
        # node-major [128, NTOWN, D]: xs[p, g, :] = x[lo + g*128 + p]
        xs = np.ascontiguousarray(
            xsh.reshape(NTOWN, 128, D).transpose(1, 0, 2).reshape(
                128, NTOWN * D))
        in_maps.append({
            "xs": xs, "deg": deg_tiles[c],
            "gidx": gidx_all[c], "rlo": rlo_all[c], "rhi": rhi_all[c],
            "W1": W1bf, "W2": W2bf, "b1bc": b1bc, "b2bc": b2bc,
            "woutbc": woutbc, "boutbc": boutbc, "identbf": identbf,
            "iotax": iotax,
        })

    res = bass_utils.run_bass_kernel_spmd(
        nc, in_maps, core_ids=list(range(NCORES)), trace=_trace)

    out = np.zeros(N, np.float32)
    for c in range(NCORES):
        o = res.results[c]["out"]  # [128, NTOWN]
        arr = o.T.ravel()          # node-major: g*128 + p
        lo, hi = c * NSH, min((c + 1) * NSH, N)
        if hi > lo:
            out[lo:hi] = arr[:hi - lo]
    if _trace:
        return out, res.exec_time_ns
    return out


# revision 24
# speedup vs baseline: 1.0844x; 1.0844x over previous
"""Distributed GCN (2-layer + readout) on 8 Trainium2 NeuronCores.

Src-sharded gather + one-hot-matmul aggregation + bf16 ReduceScatter:

Nodes are sharded 8-way by SRC owner (contiguous blocks of NSH=12544,
tile-aligned). Each GCN layer's linear W is folded to AFTER the
collective (A @ (hW) == (A @ h) @ W), so the gather table is just
u = dinv * h — elementwise, no matmul on the critical path before
gathering. Tables are stored as 256B rows (64 bf16 payload + 64 bf16
junk) to satisfy dma_gather's 256B-multiple elem constraint; gathers
are purely core-local (no AllGather).

Messages (edges) are sorted by dst and packed into 128-message columns
within tile-PAIRS (2x128 dst nodes); per pair the column count is the
max over cores so the instruction schedule is core-uniform. Each column
is aggregated with 1-2 TensorE matmuls: lhsT = a one-hot selection
matrix S (built on DVE via is_equal against an iota row; bf16) and
rhs = the gathered messages; PSUM accumulates per dst tile across a
bank of 8 tiles, then is copied (cast bf16) and spilled to a DRAM
accumulator. One bf16 ReduceScatter per layer replaces the baseline's
two f32 AllGathers; the received shard gets self-loop + @W + epilogue
via PE transposes (engines otherwise idle). This removes the combine
gather stage (~100k descriptors/layer) and the AllGather-before-gather
barrier that dominated the scatter/gather baseline.
"""
import numpy as np
import ml_dtypes

from concourse import bass, bacc, tile, mybir, bass_utils

F32 = mybir.dt.float32
BF16 = mybir.dt.bfloat16
I16 = mybir.dt.int16
I32 = mybir.dt.int32
NPBF16 = ml_dtypes.bfloat16

NCORES = 8
D = 64
GB = 64          # columns per gather block (8192 descs/instr)
CB = 32          # columns per S-build op


def _roundup(x, m):
    return (x + m - 1) // m * m


def preprocess(edge_index, n_nodes):
    src = np.asarray(edge_index[0], dtype=np.int64)
    dst = np.asarray(edge_index[1], dtype=np.int64)
    E = len(src)

    NSH = _roundup((n_nodes + NCORES - 1) // NCORES, 256)  # shard size
    assert NSH * NCORES >= n_nodes and NSH < 32768
    NTOWN = NSH // 128            # own tiles per core (even)
    NT = NCORES * NTOWN           # global dst tiles
    NPAIR = NT // 2
    assert NPAIR % 8 == 0

    owner = src // NSH
    srow = src % NSH
    pair = dst // 256
    rel = dst - pair * 256        # [0, 256)

    deg = np.bincount(dst, minlength=NSH * NCORES).astype(np.float32) + 1.0
    deg_tiles = []
    for c in range(NCORES):
        d = np.ones(NSH, np.float32)
        lo = c * NSH
        d[:] = deg[lo:lo + NSH]
        d[max(0, n_nodes - lo):] = 1.0  # pad nodes
        deg_tiles.append(np.ascontiguousarray(d.reshape(NTOWN, 128).T))

    # per (core, pair) message lists, sorted by dst
    order = np.lexsort((dst, pair, owner))
    so, sp, sr, sl = owner[order], pair[order], srow[order], rel[order]
    # counts per core,pair
    cnt = np.zeros((NCORES, NPAIR), np.int64)
    np.add.at(cnt, (so, sp), 1)
    ncols = np.maximum(1, (cnt.max(axis=0) + 127) // 128)  # per pair
    col_base = np.concatenate([[0], np.cumsum(ncols)])
    NCOL = int(col_base[-1])
    NPOS = NCOL * 128

    # fill per-core vals (gather idx) and rel arrays
    vals = np.zeros((NCORES, NPOS), np.int16)
    rlo = np.full((NCORES, NPOS), -1.0, np.float32)
    # start offsets of each (core,pair) run inside the sorted arrays
    run_start = np.zeros((NCORES, NPAIR + 1), np.int64)
    csum = np.cumsum(cnt, axis=1)
    run_start[:, 1:] = csum
    core_off = np.concatenate([[0], np.cumsum(csum[:, -1])])[:-1]
    for c in range(NCORES):
        for k in range(NPAIR):
            a = core_off[c] + run_start[c, k]
            b = core_off[c] + run_start[c, k + 1]
            m = b - a
            if m == 0:
                continue
            q = np.arange(m) + col_base[k] * 128
            vals[c, q] = sr[a:b]
            rlo[c, q] = sl[a:b]

    # crossing flags per column (shared across cores)
    rel_by_col = rlo.reshape(NCORES, NCOL, 128)
    crossing = (rel_by_col >= 128.0).any(axis=(0, 2))
    # force a crossing col on pairs whose odd tile would otherwise never
    # be written (pair empty or no rel>=128 anywhere)
    for k in range(NPAIR):
        c0, c1 = col_base[k], col_base[k + 1]
        if not crossing[c0:c1].any():
            crossing[c1 - 1] = True
    xid_of = np.full(NCOL, -1, np.int64)
    xid_of[crossing] = np.arange(int(crossing.sum()))
    NX = int(crossing.sum())

    # compact hi-rel arrays
    rhi = np.full((NCORES, NX, 128), -1.0, np.float32)
    for c in range(NCORES):
        rb = rel_by_col[c][crossing]  # [NX, 128]
        hi = rb >= 128.0
        rhi[c][hi] = rb[hi] - 128.0

    # pack gather idx: [128, NPOS//16] int16, 16-partition wrap replicated
    gidx_all, rlo_all, rhi_all = [], [], []
    for c in range(NCORES):
        gidx_all.append(np.ascontiguousarray(
            np.tile(vals[c].reshape(-1, 16).T, (8, 1))))
        rlo_all.append(np.ascontiguousarray(
            rel_by_col[c].T).astype(NPBF16))     # [128, NCOL]
        if NX:
            rhi_all.append(np.ascontiguousarray(
                rhi[c].T).astype(NPBF16))        # [128, NX]
        else:
            rhi_all.append(np.zeros((128, 1), NPBF16))

    # column schedule (shared): for each column: pair k, first/last col
    # flags, xid (or -1). The pair's matmuls form ONE psum accumulation
    # group (2KB zero region = bank): start on first col's even matmul,
    # stop on the last emitted matmul of the pair.
    cols = []
    for k in range(NPAIR):
        c0, c1 = col_base[k], col_base[k + 1]
        for c in range(c0, c1):
            last = c == c1 - 1
            xid = int(xid_of[c]) if crossing[c] else -1
            cols.append(dict(
                k=k, first=(c == c0), last=last, xid=xid,
                stop_even=(last and xid < 0),
                stop_odd=(last and xid >= 0),
            ))

    meta = dict(NSH=NSH, NTOWN=NTOWN, NT=NT, NPAIR=NPAIR, NCOL=NCOL,
                NPOS=NPOS, NX=max(NX, 1), cols=cols, n_nodes=n_nodes)
    return meta, gidx_all, rlo_all, rhi_all, deg_tiles


def build(meta):
    NSH, NTOWN, NT = meta["NSH"], meta["NTOWN"], meta["NT"]
    NCOL, NPOS, NX = meta["NCOL"], meta["NPOS"], meta["NX"]
    NPAIR = meta["NPAIR"]
    cols = meta["cols"]

    nc = bacc.Bacc("TRN2", target_bir_lowering=False, debug=False,
                   num_devices=NCORES, num_swdge_queues=4)

    xse = nc.dram_tensor("xs", [128, NTOWN * D], F32, kind="ExternalInput")
    dege = nc.dram_tensor("deg", [128, NTOWN], F32, kind="ExternalInput")
    gidxe = nc.dram_tensor("gidx", [128, NPOS // 16], I16,
                           kind="ExternalInput")
    rloe = nc.dram_tensor("rlo", [128, NCOL], BF16, kind="ExternalInput")
    rhie = nc.dram_tensor("rhi", [128, NX], BF16, kind="ExternalInput")
    W1e = nc.dram_tensor("W1", [D, D], BF16, kind="ExternalInput")
    W2e = nc.dram_tensor("W2", [D, D], BF16, kind="ExternalInput")
    b1e = nc.dram_tensor("b1bc", [128, D], F32, kind="ExternalInput")
    b2e = nc.dram_tensor("b2bc", [128, D], F32, kind="ExternalInput")
    woute = nc.dram_tensor("woutbc", [128, D], F32, kind="ExternalInput")
    boute = nc.dram_tensor("boutbc", [128, 1], F32, kind="ExternalInput")
    idente = nc.dram_tensor("identbf", [128, 128], BF16,
                            kind="ExternalInput")
    iotaxe = nc.dram_tensor("iotax", [128, 128 * CB], BF16,
                            kind="ExternalInput")
    oute = nc.dram_tensor("out", [128, NTOWN], F32, kind="ExternalOutput")

    T0 = nc.dram_tensor("T0", [NSH, 128], BF16)
    T1 = nc.dram_tensor("T1", [NSH, 128], BF16)
    # pair-major accumulator: row = k*256 + p*2 + e  (pair, lane, tile)
    acc = nc.dram_tensor("acc", [NPAIR * 256, D], BF16)
    shard = [nc.dram_tensor(f"shard{L}", [NSH, D], BF16)
             for L in (0, 1)]

    def nodemaj(dram, g):
        # DRAM [(g p), d] viewed as [128, g, d]
        return dram.ap().rearrange("(g p) d -> p g d", p=128)

    def pairmaj(apx):
        # DRAM [(k p e), d] viewed as [128, k, e, d]
        return apx.rearrange("(k p e) d -> p k e d", p=128, e=2)

    with tile.TileContext(nc) as tc:
        with (
            tc.tile_pool(name="const", bufs=1) as pool,
            tc.tile_pool(name="msg", bufs=3) as msgpool,
            tc.tile_pool(name="slo", bufs=2) as slopool,
            tc.tile_pool(name="shi", bufs=2) as shipool,
            tc.tile_pool(name="stage", bufs=4) as stagepool,
            tc.tile_pool(name="aggT", bufs=2) as aggTpool,
            tc.tile_pool(name="hg", bufs=2) as hgpool,
            tc.tile_pool(name="aggps", bufs=4, space="PSUM") as aggps,
            tc.tile_pool(name="trps", bufs=2, space="PSUM") as trps,
            tc.tile_pool(name="mmps", bufs=2, space="PSUM") as mmps,
        ):
            # ---- constants ----
            gidx_t = pool.tile([128, NPOS // 16], I16, tag="gidx")
            rlo_t = pool.tile([128, NCOL], BF16, tag="rlo")
            rhi_t = pool.tile([128, NX], BF16, tag="rhi")
            nc.scalar.dma_start(out=gidx_t[:], in_=gidxe[:])
            nc.scalar.dma_start(out=rlo_t[:], in_=rloe[:])
            nc.scalar.dma_start(out=rhi_t[:], in_=rhie[:])
            W1_t = pool.tile([D, D], BF16, tag="w1")
            W2_t = pool.tile([D, D], BF16, tag="w2")
            b1_t = pool.tile([128, D], F32, tag="b1")
            b2_t = pool.tile([128, D], F32, tag="b2")
            wout_t = pool.tile([128, D], F32, tag="wout")
            bout_t = pool.tile([128, 1], F32, tag="bout")
            ident_t = pool.tile([128, 128], BF16, tag="ident")
            nc.scalar.dma_start(out=W1_t[:], in_=W1e[:])
            nc.scalar.dma_start(out=W2_t[:], in_=W2e[:])
            nc.scalar.dma_start(out=b1_t[:], in_=b1e[:])
            nc.scalar.dma_start(out=b2_t[:], in_=b2e[:])
            nc.scalar.dma_start(out=wout_t[:], in_=woute[:])
            nc.scalar.dma_start(out=bout_t[:], in_=boute[:])
            nc.scalar.dma_start(out=ident_t[:], in_=idente[:])
            deg_t = pool.tile([128, NTOWN], F32, tag="deg")
            nc.sync.dma_start(out=deg_t[:], in_=dege[:])
            dinv_t = pool.tile([128, NTOWN], F32, tag="dinv")
            nc.scalar.activation(dinv_t[:], deg_t[:],
                                 mybir.ActivationFunctionType.Sqrt)
            nc.vector.reciprocal(dinv_t[:], dinv_t[:])
            # iota_x[p, j, c] = j  (host-precomputed; inner strides all 1
            # so the is_equal S-build hits the DVE 2x bf16 fast path)
            iota_x = pool.tile([128, 128, CB], BF16, tag="iotax")
            nc.scalar.dma_start(
                out=iota_x[:],
                in_=iotaxe.ap().rearrange("p (j c) -> p j c", c=CB))

            xs_t = pool.tile([128, NTOWN, D], F32, tag="xs")
            nc.sync.dma_start(
                out=xs_t[:],
                in_=xse.ap().rearrange("p (g d) -> p g d", d=D))

            # ---- u0 = dinv * x into junk-row staging ----
            tjunk = pool.tile([128, NTOWN, 128], BF16, tag="tjunk")
            nc.vector.memset(tjunk[:], 0.0)
            dvb = dinv_t[:].unsqueeze(2).broadcast_to([128, NTOWN, D])
            nc.vector.tensor_tensor(tjunk[:, :, 0:D], xs_t[:], dvb,
                                    mybir.AluOpType.mult)
            nc.sync.dma_start(out=nodemaj(T0, NTOWN), in_=tjunk[:])

            NSB = (NCOL + CB - 1) // CB   # S-build sub-blocks
            NGB = (NCOL + GB - 1) // GB   # gather blocks

            def agg_layer(L):
                Tbl = T0 if L == 0 else T1
                psum_live = {}
                mts, sls, shs, x0s = {}, {}, {}, {}
                for gb in range(NGB):
                    g0 = gb * GB
                    gbc = min(GB, NCOL - g0)
                    mt = msgpool.tile([128, GB, 128], BF16, tag="m")
                    nc.gpsimd.dma_gather(
                        mt[:, :gbc, :], Tbl[:],
                        gidx_t[:, g0 * 8:(g0 + gbc) * 8],
                        num_idxs=gbc * 128, num_idxs_reg=gbc * 128,
                        elem_size=128, single_packet=False,
                        queue_num=gb % 4)
                    mts[gb] = mt
                    for sb in range(g0 // CB, (g0 + gbc + CB - 1) // CB):
                        c0 = sb * CB
                        bc = min(CB, NCOL - c0)
                        # S_T[p, j, c] = (rel[p, c] == j), built transposed
                        # so every operand's inner stride is 1 (2x path)
                        sl = slopool.tile([128, 128, CB], BF16, tag="sl")
                        nc.vector.tensor_tensor(
                            sl[:, :, :bc], iota_x[:, :, :bc],
                            rlo_t[:, c0:c0 + bc].unsqueeze(1).broadcast_to(
                                [128, 128, bc]),
                            mybir.AluOpType.is_equal)
                        sls[sb] = sl
                        xids = [cols[c]["xid"] for c in range(c0, c0 + bc)
                                if cols[c]["xid"] >= 0]
                        if xids:
                            x0, nxb = xids[0], len(xids)
                            assert xids == list(range(x0, x0 + nxb))
                            x0s[sb] = x0
                            sh = shipool.tile([128, 128, CB], BF16,
                                              tag="sh")
                            nc.vector.tensor_tensor(
                                sh[:, :, :nxb], iota_x[:, :, :nxb],
                                rhi_t[:, x0:x0 + nxb].unsqueeze(1)
                                .broadcast_to([128, 128, nxb]),
                                mybir.AluOpType.is_equal)
                            shs[sb] = sh
                    for c in range(g0, g0 + gbc):
                        info = cols[c]
                        k = info["k"]
                        bank = k // 4
                        slot = k % 4
                        bank_last = slot == 3 or k == NPAIR - 1
                        if bank not in psum_live:
                            psum_live[bank] = aggps.tile(
                                [128, 8, 64], F32, tag="agg", name="aggb")
                        pt = psum_live[bank]
                        j = c - g0
                        sb = c // CB
                        nc.tensor.matmul(
                            pt[:, 2 * slot, :], sls[sb][:, :, c - sb * CB],
                            mt[:, j, 0:D],
                            start=info["first"] and slot == 0,
                            stop=info["stop_even"] and bank_last)
                        if info["xid"] >= 0:
                            xj = info["xid"] - x0s[sb]
                            nc.tensor.matmul(
                                pt[:, 2 * slot + 1, :],
                                shs[sb][:, :, xj], mt[:, j, 0:D],
                                start=False,
                                stop=info["stop_odd"] and bank_last)
                        if info["last"] and bank_last:
                            st = stagepool.tile([128, 8, 64], BF16,
                                                tag="st")
                            nc.vector.tensor_copy(st[:], pt[:])
                            k0 = bank * 4
                            nc.sync.dma_start(
                                out=pairmaj(acc[k0 * 256:(k0 + 4) * 256, :]),
                                in_=st[:].rearrange("p (k e) d -> p k e d",
                                                    e=2))
                            del psum_live[bank]
                assert not psum_live, psum_live.keys()

            def post_layer(L):
                """RS result -> agg_u -> @W -> epilogue."""
                sh_t = pool.tile([128, NTOWN, D], BF16, tag="shards")
                nc.sync.dma_start(
                    out=sh_t[:].rearrange("p (k e) d -> p k e d", e=2),
                    in_=pairmaj(shard[L].ap()))
                # add self-loop term u_own (in tjunk payload half)
                nc.vector.tensor_tensor(sh_t[:], sh_t[:],
                                        tjunk[:, :, 0:D],
                                        mybir.AluOpType.add)
                W_t = W1_t if L == 0 else W2_t
                b_t = b1_t if L == 0 else b2_t
                if L == 1:
                    o_t = pool.tile([128, NTOWN], F32, tag="o")
                for tg in range(0, NTOWN, 8):
                    ng = min(8, NTOWN - tg)
                    tp = trps.tile([64, 8, 128], BF16, tag="tr")
                    for t in range(tg, tg + ng):
                        nc.tensor.transpose(tp[:, t - tg, :], sh_t[:, t, :],
                                            ident_t[:])
                    aT = aggTpool.tile([64, 8, 128], BF16, tag="aT")
                    nc.vector.tensor_copy(aT[:, :ng, :], tp[:, :ng, :])
                    mp = mmps.tile([128, 8, 64], F32, tag="mm")
                    for t in range(tg, tg + ng):
                        nc.tensor.matmul(mp[:, t - tg, :], aT[:, t - tg, :],
                                         W_t[:])
                    hg = hgpool.tile([128, 8, 64], F32, tag="hg")
                    dvg = dinv_t[:, tg:tg + ng].unsqueeze(2).broadcast_to(
                        [128, ng, 64])
                    bbg = b_t[:].unsqueeze(1).broadcast_to([128, ng, 64])
                    nc.vector.tensor_tensor(hg[:, :ng, :], mp[:, :ng, :],
                                            dvg, mybir.AluOpType.mult)
                    nc.vector.tensor_tensor(hg[:, :ng, :], hg[:, :ng, :],
                                            bbg, mybir.AluOpType.add)
                    nc.scalar.activation(hg[:, :ng, :], hg[:, :ng, :],
                                         mybir.ActivationFunctionType.Relu)
                    if L == 0:
                        # u1 = dinv * h into table staging payload
                        nc.vector.tensor_tensor(
                            tjunk[:, tg:tg + ng, 0:D], hg[:, :ng, :], dvg,
                            mybir.AluOpType.mult)
                    else:
                        wbg = wout_t[:].unsqueeze(1).broadcast_to(
                            [128, ng, 64])
                        nc.vector.tensor_tensor(hg[:, :ng, :], hg[:, :ng, :],
                                                wbg, mybir.AluOpType.mult)
                        nc.vector.tensor_reduce(
                            o_t[:, tg:tg + ng], hg[:, :ng, :],
                            axis=mybir.AxisListType.X,
                            op=mybir.AluOpType.add)
                if L == 0:
                    nc.sync.dma_start(out=nodemaj(T1, NTOWN), in_=tjunk[:])
                else:
                    nc.vector.tensor_scalar_add(o_t[:], o_t[:], bout_t[:])
                    nc.sync.dma_start(out=oute[:], in_=o_t[:])

            for L in (0, 1):
                agg_layer(L)
                nc.gpsimd.collective_compute(
                    "ReduceScatter", mybir.AluOpType.add,
                    replica_groups=[list(range(NCORES))],
                    ins=[acc.ap().opt()],
                    outs=[shard[L].ap().opt()])
                post_layer(L)

    nc.compile()
    return nc


_CACHE = {}


def kernel(x, edge_index, batch, W1, b1, W2, b2, Wout, bout, _trace=False):
    x = np.asarray(x, np.float32)
    edge_index = np.asarray(edge_index)
    W1 = np.asarray(W1, np.float32)
    W2 = np.asarray(W2, np.float32)
    b1 = np.asarray(b1, np.float32)
    b2 = np.asarray(b2, np.float32)
    Wout = np.asarray(Wout, np.float32)
    bout = np.asarray(bout, np.float32).reshape(-1)
    N = x.shape[0]

    key = (N, edge_index.shape[1])
    if key not in _CACHE:
        meta, gidx_all, rlo_all, rhi_all, deg_tiles = preprocess(
            edge_index, N)
        nc = build(meta)
        _CACHE[key] = (meta, gidx_all, rlo_all, rhi_all, deg_tiles, nc)
    meta, gidx_all, rlo_all, rhi_all, deg_tiles, nc = _CACHE[key]
    NSH, NTOWN = meta["NSH"], meta["NTOWN"]

    identbf = np.eye(128, dtype=np.float32).astype(NPBF16)
    iotax = np.ascontiguousarray(np.broadcast_to(
        np.arange(128, dtype=np.float32)[None, :, None],
        (128, 128, CB)).reshape(128, 128 * CB)).astype(NPBF16)
    b1bc = np.tile(b1[None, :], (128, 1)).astype(np.float32)
    b2bc = np.tile(b2[None, :], (128, 1)).astype(np.float32)
    woutbc = np.tile(Wout.reshape(1, -1), (128, 1)).astype(np.float32)
    boutbc = np.full((128, 1), float(bout[0]), np.float32)
    W1bf = W1.astype(NPBF16)
    W2bf = W2.astype(NPBF16)

    in_maps = []
    for c in range(NCORES):
        xsh = np.zeros((NSH, D), np.float32)
        lo, hi = c * NSH, min((c + 1) * NSH, N)
        if hi > lo:
            xsh[:hi - lo] = x[lo:hi]
        # node-major [128, NTOWN, D]: xs[p, g, :] = x[lo + g*128 + p]
        xs = np.ascontiguousarray(
            xsh.reshape(NTOWN, 128, D).transpose(1, 0, 2).reshape(
                128, NTOWN * D))
        in_maps.append({
            "xs": xs, "deg": deg_tiles[c],
            "gidx": gidx_all[c], "rlo": rlo_all[c], "rhi": rhi_all[c],
            "W1": W1bf, "W2": W2bf, "b1bc": b1bc, "b2bc": b2bc,
            "woutbc": woutbc, "boutbc": boutbc, "identbf": identbf,
            "iotax": iotax,
        })

    res = bass_utils.run_bass_kernel_spmd(
        nc, in_maps, core_ids=list(range(NCORES)), trace=_trace)

    out = np.zeros(N, np.float32)
    for c in range(NCORES):
        o = res.results[c]["out"]  # [128, NTOWN]
        arr = o.T.ravel()          # node-major: g*128 + p
        lo, hi = c * NSH, min((c + 1) * NSH, N)
        if hi > lo:
            out[lo:hi] = arr[:hi - lo]
    if _trace:
        return out, res.exec_time_ns
    return out


# revision 30
# speedup vs baseline: 1.2048x; 1.1111x over previous
"""Distributed GCN (2-layer + readout) on 8 Trainium2 NeuronCores.

Src-sharded gather + one-hot-matmul aggregation + bf16 ReduceScatter:

Nodes are sharded 8-way by SRC owner (contiguous blocks of NSH,
tile-aligned). Each GCN layer's linear W is folded to AFTER the
collective (A @ (hW) == (A @ h) @ W), so the gather table is just
u = dinv * h — elementwise, no matmul before gathering. Tables are
256B rows (64 bf16 payload + 64 junk) to satisfy dma_gather's 256B
elem constraint; gathers are purely core-local (no AllGather).

Messages (edges) are sorted by dst and packed into 128-message columns
within tile-PAIRS (256 dst nodes); per pair the column count is the
max over cores so the schedule is core-uniform. Each column costs 1-2
TensorE matmuls: lhsT = a one-hot selection matrix S (DVE is_equal
against a pre-expanded iota, all inner strides 1 -> 2x bf16 mode) and
rhs = the gathered messages; PSUM accumulates banks of <=4 pairs (one
accumulation group per 2KB zero region), cast to bf16 and spilled to a
pair-major DRAM accumulator. Pairs are emitted half-by-half (half A =
first 25 pairs of every core's shard) so each half's ReduceScatter and
post-pass (self-loop + @W + epilogue via PE transposes) overlap the
other half's aggregation. One bf16 RS per half replaces the baseline's
f32 AllGathers-before-gather barrier.
"""
import numpy as np
import ml_dtypes

from concourse import bass, bacc, tile, mybir, bass_utils

F32 = mybir.dt.float32
BF16 = mybir.dt.bfloat16
I16 = mybir.dt.int16
NPBF16 = ml_dtypes.bfloat16

NCORES = 8
D = 64
GB = 32          # columns per gather instr
CB = 32          # columns per S-build op


def _roundup(x, m):
    return (x + m - 1) // m * m


def preprocess(edge_index, n_nodes):
    src = np.asarray(edge_index[0], dtype=np.int64)
    dst = np.asarray(edge_index[1], dtype=np.int64)

    NSH = _roundup((n_nodes + NCORES - 1) // NCORES, 256)  # shard size
    assert NSH * NCORES >= n_nodes and NSH < 32768
    NTOWN = NSH // 128            # own tiles per core (even)
    NT = NCORES * NTOWN           # global dst tiles
    NPAIR = NT // 2
    KOWN = NTOWN // 2             # pairs per core's dst shard
    KA = (KOWN + 1) // 2          # half A pairs per core
    # halves of the dst space (by local pair id)
    half_of_k = (np.arange(NPAIR) % KOWN) >= KA  # 0 = A, 1 = B

    owner = src // NSH
    srow = src % NSH
    pair = dst // 256
    rel = dst - pair * 256        # [0, 256)

    deg = np.bincount(dst, minlength=NSH * NCORES).astype(np.float32) + 1.0
    deg_tiles = []
    for c in range(NCORES):
        d = np.ones(NSH, np.float32)
        lo = c * NSH
        d[:] = deg[lo:lo + NSH]
        d[max(0, n_nodes - lo):] = 1.0  # pad nodes
        deg_tiles.append(np.ascontiguousarray(d.reshape(NTOWN, 128).T))

    # emission order of pairs: half A pairs (dst-core asc, local asc),
    # then half B
    emit_pairs = []
    for h in (0, 1):
        for dc in range(NCORES):
            for kl in range(KOWN):
                if (kl >= KA) == bool(h):
                    emit_pairs.append(dc * KOWN + kl)
    emit_pairs = np.array(emit_pairs)
    emit_rank = np.empty(NPAIR, np.int64)
    emit_rank[emit_pairs] = np.arange(NPAIR)

    order = np.lexsort((dst, pair, owner))
    so, sp, sr, sl = owner[order], pair[order], srow[order], rel[order]
    cnt = np.zeros((NCORES, NPAIR), np.int64)
    np.add.at(cnt, (so, sp), 1)
    ncols_k = np.maximum(1, (cnt.max(axis=0) + 127) // 128)  # by global k
    # column base in EMISSION order
    ncols_emit = ncols_k[emit_pairs]
    col_base_emit = np.concatenate([[0], np.cumsum(ncols_emit)])
    NCOL = int(col_base_emit[-1])
    NPOS = NCOL * 128
    col_base_of_k = np.empty(NPAIR, np.int64)
    col_base_of_k[emit_pairs] = col_base_emit[:-1]

    # fill per-core vals (gather idx) and rel arrays
    vals = np.zeros((NCORES, NPOS), np.int16)
    rlo = np.full((NCORES, NPOS), -1.0, np.float32)
    csum = np.cumsum(cnt, axis=1)
    run_start = np.zeros((NCORES, NPAIR + 1), np.int64)
    run_start[:, 1:] = csum
    core_off = np.concatenate([[0], np.cumsum(csum[:, -1])])[:-1]
    for c in range(NCORES):
        for k in range(NPAIR):
            a = core_off[c] + run_start[c, k]
            b = core_off[c] + run_start[c, k + 1]
            m = b - a
            if m == 0:
                continue
            q = np.arange(m) + col_base_of_k[k] * 128
            vals[c, q] = sr[a:b]
            rlo[c, q] = sl[a:b]

    rel_by_col = rlo.reshape(NCORES, NCOL, 128)
    crossing = (rel_by_col >= 128.0).any(axis=(0, 2))
    for k in range(NPAIR):
        c0 = col_base_of_k[k]
        c1 = c0 + ncols_k[k]
        if not crossing[c0:c1].any():
            crossing[c1 - 1] = True
    xid_of = np.full(NCOL, -1, np.int64)
    xid_of[crossing] = np.arange(int(crossing.sum()))
    NX = int(crossing.sum())

    rhi = np.full((NCORES, NX, 128), -1.0, np.float32)
    for c in range(NCORES):
        rb = rel_by_col[c][crossing]
        hi = rb >= 128.0
        rhi[c][hi] = rb[hi] - 128.0

    gidx_all, rlo_all, rhi_all = [], [], []
    for c in range(NCORES):
        gidx_all.append(np.ascontiguousarray(
            np.tile(vals[c].reshape(-1, 16).T, (8, 1))))
        rlo_all.append(np.ascontiguousarray(
            rel_by_col[c].T).astype(NPBF16))
        if NX:
            rhi_all.append(np.ascontiguousarray(
                rhi[c].T).astype(NPBF16))
        else:
            rhi_all.append(np.zeros((128, 1), NPBF16))

    # column schedule in emission order. Banks = runs of <=4
    # emission-consecutive pairs with contiguous global k (never
    # crossing a (core, half) boundary). One psum accumulation group
    # (2KB zero region) per bank.
    cols = []
    banks = []          # (k0, npairs)
    ei = 0
    while ei < NPAIR:
        k0 = emit_pairs[ei]
        npk = 1
        while (npk < 4 and ei + npk < NPAIR
               and emit_pairs[ei + npk] == k0 + npk
               and (k0 % KOWN) // KA == ((k0 + npk) % KOWN) // KA
               and (k0 + npk) // KOWN == k0 // KOWN):
            npk += 1
        banks.append((int(k0), npk))
        ei += npk
    bank_of_k = {}
    for bi, (k0, npk) in enumerate(banks):
        for s in range(npk):
            bank_of_k[k0 + s] = (bi, s)
    lastA_col = -1
    for ei, k in enumerate(emit_pairs):
        c0 = int(col_base_of_k[k])
        c1 = c0 + int(ncols_k[k])
        bi, slot = bank_of_k[k]
        k0, npk = banks[bi]
        for c in range(c0, c1):
            last = c == c1 - 1
            xid = int(xid_of[c]) if crossing[c] else -1
            bank_last_pair = slot == npk - 1
            cols.append(dict(
                k=int(k), first=(c == c0), last=last, xid=xid,
                bank=bi, slot=slot,
                bank_start=(c == c0 and slot == 0),
                stop_even=(last and xid < 0 and bank_last_pair),
                stop_odd=(last and xid >= 0 and bank_last_pair),
                spill=(last and bank_last_pair),
            ))
        if not half_of_k[k]:
            lastA_col = len(cols) - 1

    meta = dict(NSH=NSH, NTOWN=NTOWN, NT=NT, NPAIR=NPAIR, NCOL=NCOL,
                NPOS=NPOS, NX=max(NX, 1), cols=cols, banks=banks,
                KOWN=KOWN, KA=KA, lastA_col=lastA_col, n_nodes=n_nodes)
    return meta, gidx_all, rlo_all, rhi_all, deg_tiles


def build(meta):
    NSH, NTOWN, NT = meta["NSH"], meta["NTOWN"], meta["NT"]
    NCOL, NPOS, NX = meta["NCOL"], meta["NPOS"], meta["NX"]
    NPAIR, KOWN, KA = meta["NPAIR"], meta["KOWN"], meta["KA"]
    cols, banks = meta["cols"], meta["banks"]
    lastA_col = meta["lastA_col"]
    GA = 2 * KA          # own tiles in half A
    NGRP = (NTOWN + 7) // 8
    GRPA = GA // 8       # full 8-tile post groups entirely inside half A

    nc = bacc.Bacc("TRN2", target_bir_lowering=False, debug=False,
                   num_devices=NCORES, num_swdge_queues=4)

    xse = nc.dram_tensor("xs", [128, NTOWN * D], F32, kind="ExternalInput")
    dege = nc.dram_tensor("deg", [128, NTOWN], F32, kind="ExternalInput")
    gidxe = nc.dram_tensor("gidx", [128, NPOS // 16], I16,
                           kind="ExternalInput")
    rloe = nc.dram_tensor("rlo", [128, NCOL], BF16, kind="ExternalInput")
    rhie = nc.dram_tensor("rhi", [128, NX], BF16, kind="ExternalInput")
    W1e = nc.dram_tensor("W1", [D, D], BF16, kind="ExternalInput")
    W2e = nc.dram_tensor("W2", [D, D], BF16, kind="ExternalInput")
    b1e = nc.dram_tensor("b1bc", [128, D], F32, kind="ExternalInput")
    b2e = nc.dram_tensor("b2bc", [128, D], F32, kind="ExternalInput")
    woute = nc.dram_tensor("woutbc", [128, D], F32, kind="ExternalInput")
    boute = nc.dram_tensor("boutbc", [128, 1], F32, kind="ExternalInput")
    idente = nc.dram_tensor("identbf", [128, 128], BF16,
                            kind="ExternalInput")
    iotaxe = nc.dram_tensor("iotax", [128, 128 * CB], BF16,
                            kind="ExternalInput")
    oute = nc.dram_tensor("out", [128, NTOWN], F32, kind="ExternalOutput")

    T0 = nc.dram_tensor("T0", [NSH, 128], BF16)
    T1 = nc.dram_tensor("T1", [NSH, 128], BF16)
    # pair-major accumulators, one per dst half:
    # accA row = (dstcore*KA + kl)*256 + p*2 + e
    KB = KOWN - KA
    accH = [nc.dram_tensor("accA", [NCORES * KA * 256, D], BF16),
            nc.dram_tensor("accB", [NCORES * KB * 256, D], BF16)]
    shardH = [[nc.dram_tensor(f"shard{L}{h}",
                              [(KA if h == 0 else KB) * 256, D], BF16)
               for h in (0, 1)] for L in (0, 1)]

    def pairmaj(apx, e=2):
        # DRAM [(k p e), d] viewed as [128, k, e, d]
        return apx.rearrange("(k p e) d -> p k e d", p=128, e=e)

    with tile.TileContext(nc) as tc:
        with (
            tc.tile_pool(name="const", bufs=1) as pool,
            tc.tile_pool(name="msg", bufs=6) as msgpool,
            tc.tile_pool(name="slo", bufs=3) as slopool,
            tc.tile_pool(name="shi", bufs=3) as shipool,
            tc.tile_pool(name="stage", bufs=4) as stagepool,
            tc.tile_pool(name="aggT", bufs=2) as aggTpool,
            tc.tile_pool(name="hg", bufs=2) as hgpool,
            tc.tile_pool(name="aggps", bufs=4, space="PSUM") as aggps,
            tc.tile_pool(name="trps", bufs=2, space="PSUM") as trps,
            tc.tile_pool(name="mmps", bufs=2, space="PSUM") as mmps,
        ):
            # ---- constants ----
            gidx_t = pool.tile([128, NPOS // 16], I16, tag="gidx")
            rlo_t = pool.tile([128, NCOL], BF16, tag="rlo")
            rhi_t = pool.tile([128, NX], BF16, tag="rhi")
            nc.scalar.dma_start(out=gidx_t[:], in_=gidxe[:])
            nc.scalar.dma_start(out=rlo_t[:], in_=rloe[:])
            nc.scalar.dma_start(out=rhi_t[:], in_=rhie[:])
            W1_t = pool.tile([D, D], BF16, tag="w1")
            W2_t = pool.tile([D, D], BF16, tag="w2")
            b1_t = pool.tile([128, D], F32, tag="b1")
            b2_t = pool.tile([128, D], F32, tag="b2")
            wout_t = pool.tile([128, D], F32, tag="wout")
            bout_t = pool.tile([128, 1], F32, tag="bout")
            ident_t = pool.tile([128, 128], BF16, tag="ident")
            nc.scalar.dma_start(out=W1_t[:], in_=W1e[:])
            nc.scalar.dma_start(out=W2_t[:], in_=W2e[:])
            nc.scalar.dma_start(out=b1_t[:], in_=b1e[:])
            nc.scalar.dma_start(out=b2_t[:], in_=b2e[:])
            nc.scalar.dma_start(out=wout_t[:], in_=woute[:])
            nc.scalar.dma_start(out=bout_t[:], in_=boute[:])
            nc.scalar.dma_start(out=ident_t[:], in_=idente[:])
            deg_t = pool.tile([128, NTOWN], F32, tag="deg")
            nc.sync.dma_start(out=deg_t[:], in_=dege[:])
            dinv_t = pool.tile([128, NTOWN], F32, tag="dinv")
            nc.scalar.activation(dinv_t[:], deg_t[:],
                                 mybir.ActivationFunctionType.Sqrt)
            nc.vector.reciprocal(dinv_t[:], dinv_t[:])
            # iota_x[p, j, c] = j (all inner strides 1 for 2x is_equal)
            iota_x = pool.tile([128, 128, CB], BF16, tag="iotax")
            nc.scalar.dma_start(
                out=iota_x[:],
                in_=iotaxe.ap().rearrange("p (j c) -> p j c", c=CB))

            xs_t = pool.tile([128, NTOWN, D], F32, tag="xs")
            nc.sync.dma_start(
                out=xs_t[:],
                in_=xse.ap().rearrange("p (g d) -> p g d", d=D))

            # ---- u0 = dinv * x into junk-row staging ----
            tjunk = pool.tile([128, NTOWN, 128], BF16, tag="tjunk")
            nc.vector.memset(tjunk[:], 0.0)
            dvb = dinv_t[:].unsqueeze(2).broadcast_to([128, NTOWN, D])
            nc.vector.tensor_tensor(tjunk[:, :, 0:D], xs_t[:], dvb,
                                    mybir.AluOpType.mult)
            nc.sync.dma_start(
                out=T0.ap().rearrange("(g p) d -> p g d", p=128),
                in_=tjunk[:])

            NGB = (NCOL + GB - 1) // GB

            def rs_half(L, h):
                nc.gpsimd.collective_compute(
                    "ReduceScatter", mybir.AluOpType.add,
                    replica_groups=[list(range(NCORES))],
                    ins=[accH[h].ap().opt()],
                    outs=[shardH[L][h].ap().opt()])

            def agg_layer(L):
                Tbl = T0 if L == 0 else T1
                psum_live = {}
                sls, shs, x0s = {}, {}, {}
                for gb in range(NGB):
                    g0 = gb * GB
                    gbc = min(GB, NCOL - g0)
                    mt = msgpool.tile([128, GB, 128], BF16, tag="m")
                    nc.gpsimd.dma_gather(
                        mt[:, :gbc, :], Tbl[:],
                        gidx_t[:, g0 * 8:(g0 + gbc) * 8],
                        num_idxs=gbc * 128, num_idxs_reg=gbc * 128,
                        elem_size=128, single_packet=False,
                        queue_num=gb % 4)
                    for sb in range(g0 // CB,
                                    (g0 + gbc + CB - 1) // CB):
                        c0 = sb * CB
                        bc = min(CB, NCOL - c0)
                        sl = slopool.tile([128, 128, CB], BF16, tag="sl")
                        nc.vector.tensor_tensor(
                            sl[:, :, :bc], iota_x[:, :, :bc],
                            rlo_t[:, c0:c0 + bc].unsqueeze(1).broadcast_to(
                                [128, 128, bc]),
                            mybir.AluOpType.is_equal)
                        sls[sb] = sl
                        xids = [cols[c]["xid"] for c in range(c0, c0 + bc)
                                if cols[c]["xid"] >= 0]
                        if xids:
                            x0, nxb = xids[0], len(xids)
                            assert xids == list(range(x0, x0 + nxb))
                            x0s[sb] = x0
                            sh = shipool.tile([128, 128, CB], BF16,
                                              tag="sh")
                            nc.vector.tensor_tensor(
                                sh[:, :, :nxb], iota_x[:, :, :nxb],
                                rhi_t[:, x0:x0 + nxb].unsqueeze(1)
                                .broadcast_to([128, 128, nxb]),
                                mybir.AluOpType.is_equal)
                            shs[sb] = sh
                    for c in range(g0, g0 + gbc):
                        info = cols[c]
                        bi = info["bank"]
                        slot = info["slot"]
                        if bi not in psum_live:
                            psum_live[bi] = aggps.tile(
                                [128, 8, 64], F32, tag="agg", name="aggb")
                        pt = psum_live[bi]
                        j = c - g0
                        sb = c // CB
                        nc.tensor.matmul(
                            pt[:, 2 * slot, :],
                            sls[sb][:, :, c - sb * CB], mt[:, j, 0:D],
                            start=info["bank_start"],
                            stop=info["stop_even"])
                        if info["xid"] >= 0:
                            xj = info["xid"] - x0s[sb]
                            nc.tensor.matmul(
                                pt[:, 2 * slot + 1, :],
                                shs[sb][:, :, xj], mt[:, j, 0:D],
                                start=False, stop=info["stop_odd"])
                        if info["spill"]:
                            k0, npk = banks[bi]
                            dc, kl = k0 // KOWN, k0 % KOWN
                            if kl < KA:
                                av, r0 = accH[0], dc * KA + kl
                            else:
                                av, r0 = accH[1], dc * (KOWN - KA) + (
                                    kl - KA)
                            st = stagepool.tile([128, 8, 64], BF16,
                                                tag="st")
                            nc.vector.tensor_copy(st[:, :2 * npk, :],
                                                  pt[:, :2 * npk, :])
                            nc.sync.dma_start(
                                out=pairmaj(
                                    av[r0 * 256:(r0 + npk) * 256, :]),
                                in_=st[:, :2 * npk, :].rearrange(
                                    "p (k e) d -> p k e d", e=2))
                            del psum_live[bi]
                        if c == lastA_col:
                            rs_half(L, 0)
                            post_half(L, 0)
                rs_half(L, 1)
                post_half(L, 1)
                assert not psum_live, psum_live.keys()

            o_t = pool.tile([128, NTOWN], F32, tag="o")
            sh_ts = {}

            def post_half(L, h):
                """RS_h result -> agg_u -> @W -> epilogue for that half."""
                if h == 0:
                    sh_t = pool.tile([128, NTOWN, D], BF16, tag="shards",
                                     name="sh_t")
                    sh_ts[L] = sh_t
                    ga, gb_ = 0, 2 * KA
                    grps = range(0, GRPA)
                else:
                    sh_t = sh_ts[L]
                    ga, gb_ = 2 * KA, NTOWN
                    grps = range(GRPA, NGRP)
                nc.sync.dma_start(
                    out=sh_t[:, ga:gb_, :].rearrange(
                        "p (k e) d -> p k e d", e=2),
                    in_=pairmaj(shardH[L][h].ap()))
                nc.vector.tensor_tensor(sh_t[:, ga:gb_, :],
                                        sh_t[:, ga:gb_, :],
                                        tjunk[:, ga:gb_, 0:D],
                                        mybir.AluOpType.add)
                W_t = W1_t if L == 0 else W2_t
                b_t = b1_t if L == 0 else b2_t
                for tg8 in grps:
                    tg = tg8 * 8
                    ng = min(8, NTOWN - tg)
                    tp = trps.tile([64, 8, 128], BF16, tag="tr")
                    for t in range(tg, tg + ng):
                        nc.tensor.transpose(tp[:, t - tg, :],
                                            sh_t[:, t, :], ident_t[:])
                    aT = aggTpool.tile([64, 8, 128], BF16, tag="aT")
                    nc.vector.tensor_copy(aT[:, :ng, :], tp[:, :ng, :])
                    mp = mmps.tile([128, 8, 64], F32, tag="mm")
                    for t in range(tg, tg + ng):
                        nc.tensor.matmul(mp[:, t - tg, :],
                                         aT[:, t - tg, :], W_t[:])
                    hg = hgpool.tile([128, 8, 64], F32, tag="hg")
                    dvg = dinv_t[:, tg:tg + ng].unsqueeze(2).broadcast_to(
                        [128, ng, 64])
                    bbg = b_t[:].unsqueeze(1).broadcast_to([128, ng, 64])
                    nc.vector.tensor_tensor(hg[:, :ng, :], mp[:, :ng, :],
                                            dvg, mybir.AluOpType.mult)
                    nc.vector.tensor_tensor(hg[:, :ng, :], hg[:, :ng, :],
                                            bbg, mybir.AluOpType.add)
                    nc.scalar.activation(hg[:, :ng, :], hg[:, :ng, :],
                                         mybir.ActivationFunctionType.Relu)
                    if L == 0:
                        nc.vector.tensor_tensor(
                            tjunk[:, tg:tg + ng, 0:D], hg[:, :ng, :],
                            dvg, mybir.AluOpType.mult)
                    else:
                        wbg = wout_t[:].unsqueeze(1).broadcast_to(
                            [128, ng, 64])
                        nc.vector.tensor_tensor(hg[:, :ng, :],
                                                hg[:, :ng, :], wbg,
                                                mybir.AluOpType.mult)
                        nc.vector.tensor_reduce(
                            o_t[:, tg:tg + ng], hg[:, :ng, :],
                            axis=mybir.AxisListType.X,
                            op=mybir.AluOpType.add)
                if h == 1:
                    if L == 0:
                        nc.sync.dma_start(
                            out=T1.ap().rearrange("(g p) d -> p g d",
                                                  p=128),
                            in_=tjunk[:])
                    else:
                        nc.vector.tensor_scalar_add(o_t[:], o_t[:],
                                                    bout_t[:])
                        nc.sync.dma_start(out=oute[:], in_=o_t[:])

            for L in (0, 1):
                agg_layer(L)

    nc.compile()
    return nc


_CACHE = {}


def kernel(x, edge_index, batch, W1, b1, W2, b2, Wout, bout, _trace=False):
    x = np.asarray(x, np.float32)
    edge_index = np.asarray(edge_index)
    W1 = np.asarray(W1, np.float32)
    W2 = np.asarray(W2, np.float32)
    b1 = np.asarray(b1, np.float32)
    b2 = np.asarray(b2, np.float32)
    Wout = np.asarray(Wout, np.float32)
    bout = np.asarray(bout, np.float32).reshape(-1)
    N = x.shape[0]

    key = (N, edge_index.shape[1])
    if key not in _CACHE:
        meta, gidx_all, rlo_all, rhi_all, deg_tiles = preprocess(
            edge_index, N)
        nc = build(meta)
        _CACHE[key] = (meta, gidx_all, rlo_all, rhi_all, deg_tiles, nc)
    meta, gidx_all, rlo_all, rhi_all, deg_tiles, nc = _CACHE[key]
    NSH, NTOWN = meta["NSH"], meta["NTOWN"]

    identbf = np.eye(128, dtype=np.float32).astype(NPBF16)
    iotax = np.ascontiguousarray(np.broadcast_to(
        np.arange(128, dtype=np.float32)[None, :, None],
        (128, 128, CB)).reshape(128, 128 * CB)).astype(NPBF16)
    b1bc = np.tile(b1[None, :], (128, 1)).astype(np.float32)
    b2bc = np.tile(b2[None, :], (128, 1)).astype(np.float32)
    woutbc = np.tile(Wout.reshape(1, -1), (128, 1)).astype(np.float32)
    boutbc = np.full((128, 1), float(bout[0]), np.float32)
    W1bf = W1.astype(NPBF16)
    W2bf = W2.astype(NPBF16)

    in_maps = []
    for c in range(NCORES):
        xsh = np.zeros((NSH, D), np.float32)
        lo, hi = c * NSH, min((c + 1) * NSH, N)
        if hi > lo:
            xsh[:hi - lo] = x[lo:hi]
        xs = np.ascontiguousarray(
            xsh.reshape(NTOWN, 128, D).transpose(1, 0, 2).reshape(
                128, NTOWN * D))
        in_maps.append({
            "xs": xs, "deg": deg_tiles[c],
            "gidx": gidx_all[c], "rlo": rlo_all[c], "rhi": rhi_all[c],
            "W1": W1bf, "W2": W2bf, "b1bc": b1bc, "b2bc": b2bc,
            "woutbc": woutbc, "boutbc": boutbc, "identbf": identbf,
            "iotax": iotax,
        })

    res = bass_utils.run_bass_kernel_spmd(
        nc, in_maps, core_ids=list(range(NCORES)), trace=_trace)

    out = np.zeros(N, np.float32)
    for c in range(NCORES):
        o = res.results[c]["out"]  # [128, NTOWN]
        arr = o.T.ravel()          # node-major: g*128 + p
        lo, hi = c * NSH, min((c + 1) * NSH, N)
        if hi > lo:
            out[lo:hi] = arr[:hi - lo]
    if _trace:
        return out, res.exec_time_ns
    return out


# revision 31
# speedup vs baseline: 1.2412x; 1.0302x over previous
"""Distributed GCN (2-layer + readout) on 8 Trainium2 NeuronCores.

Src-sharded gather + one-hot-matmul aggregation + bf16 ReduceScatter:

Nodes are sharded 8-way by SRC owner (contiguous blocks of NSH,
tile-aligned). Each GCN layer's linear W is folded to AFTER the
collective (A @ (hW) == (A @ h) @ W), so the gather table is just
u = dinv * h — elementwise, no matmul before gathering. Tables are
256B rows (64 bf16 payload + 64 junk) to satisfy dma_gather's 256B
elem constraint; gathers are purely core-local (no AllGather).

Messages (edges) are sorted by dst and packed into 128-message columns
within tile-PAIRS (256 dst nodes); per pair the column count is the
max over cores so the schedule is core-uniform. Each column costs 1-2
TensorE matmuls: lhsT = a one-hot selection matrix S (DVE is_equal
against a pre-expanded iota, all inner strides 1 -> 2x bf16 mode) and
rhs = the gathered messages; PSUM accumulates banks of <=4 pairs (one
accumulation group per 2KB zero region), cast to bf16 and spilled to a
pair-major DRAM accumulator. Pairs are emitted half-by-half (half A =
first 25 pairs of every core's shard) so each half's ReduceScatter and
post-pass (self-loop + @W + epilogue via PE transposes) overlap the
other half's aggregation. One bf16 RS per half replaces the baseline's
f32 AllGathers-before-gather barrier.
"""
import numpy as np
import ml_dtypes

from concourse import bass, bacc, tile, mybir, bass_utils

F32 = mybir.dt.float32
BF16 = mybir.dt.bfloat16
I16 = mybir.dt.int16
NPBF16 = ml_dtypes.bfloat16

NCORES = 8
D = 64
GB = 32          # columns per gather instr
CB = 32          # columns per S-build op


def _roundup(x, m):
    return (x + m - 1) // m * m


def preprocess(edge_index, n_nodes):
    src = np.asarray(edge_index[0], dtype=np.int64)
    dst = np.asarray(edge_index[1], dtype=np.int64)

    NSH = _roundup((n_nodes + NCORES - 1) // NCORES, 256)  # shard size
    assert NSH * NCORES >= n_nodes and NSH < 32768
    NTOWN = NSH // 128            # own tiles per core (even)
    NT = NCORES * NTOWN           # global dst tiles
    NPAIR = NT // 2
    KOWN = NTOWN // 2             # pairs per core's dst shard
    KA = (KOWN + 1) // 2          # half A pairs per core
    # halves of the dst space (by local pair id)
    half_of_k = (np.arange(NPAIR) % KOWN) >= KA  # 0 = A, 1 = B

    owner = src // NSH
    srow = src % NSH
    pair = dst // 256
    rel = dst - pair * 256        # [0, 256)

    deg = np.bincount(dst, minlength=NSH * NCORES).astype(np.float32) + 1.0
    deg_tiles = []
    for c in range(NCORES):
        d = np.ones(NSH, np.float32)
        lo = c * NSH
        d[:] = deg[lo:lo + NSH]
        d[max(0, n_nodes - lo):] = 1.0  # pad nodes
        deg_tiles.append(np.ascontiguousarray(d.reshape(NTOWN, 128).T))

    # emission order of pairs: half A pairs (dst-core asc, local asc),
    # then half B
    emit_pairs = []
    for h in (0, 1):
        for dc in range(NCORES):
            for kl in range(KOWN):
                if (kl >= KA) == bool(h):
                    emit_pairs.append(dc * KOWN + kl)
    emit_pairs = np.array(emit_pairs)
    emit_rank = np.empty(NPAIR, np.int64)
    emit_rank[emit_pairs] = np.arange(NPAIR)

    order = np.lexsort((dst, pair, owner))
    so, sp, sr, sl = owner[order], pair[order], srow[order], rel[order]
    cnt = np.zeros((NCORES, NPAIR), np.int64)
    np.add.at(cnt, (so, sp), 1)
    ncols_k = np.maximum(1, (cnt.max(axis=0) + 127) // 128)  # by global k
    # column base in EMISSION order
    ncols_emit = ncols_k[emit_pairs]
    col_base_emit = np.concatenate([[0], np.cumsum(ncols_emit)])
    NCOL = int(col_base_emit[-1])
    NPOS = NCOL * 128
    col_base_of_k = np.empty(NPAIR, np.int64)
    col_base_of_k[emit_pairs] = col_base_emit[:-1]

    # fill per-core vals (gather idx) and rel arrays
    vals = np.zeros((NCORES, NPOS), np.int16)
    rlo = np.full((NCORES, NPOS), -1.0, np.float32)
    csum = np.cumsum(cnt, axis=1)
    run_start = np.zeros((NCORES, NPAIR + 1), np.int64)
    run_start[:, 1:] = csum
    core_off = np.concatenate([[0], np.cumsum(csum[:, -1])])[:-1]
    for c in range(NCORES):
        for k in range(NPAIR):
            a = core_off[c] + run_start[c, k]
            b = core_off[c] + run_start[c, k + 1]
            m = b - a
            if m == 0:
                continue
            q = np.arange(m) + col_base_of_k[k] * 128
            vals[c, q] = sr[a:b]
            rlo[c, q] = sl[a:b]

    rel_by_col = rlo.reshape(NCORES, NCOL, 128)
    crossing = (rel_by_col >= 128.0).any(axis=(0, 2))
    for k in range(NPAIR):
        c0 = col_base_of_k[k]
        c1 = c0 + ncols_k[k]
        if not crossing[c0:c1].any():
            crossing[c1 - 1] = True
    xid_of = np.full(NCOL, -1, np.int64)
    xid_of[crossing] = np.arange(int(crossing.sum()))
    NX = int(crossing.sum())

    rhi = np.full((NCORES, NX, 128), -1.0, np.float32)
    for c in range(NCORES):
        rb = rel_by_col[c][crossing]
        hi = rb >= 128.0
        rhi[c][hi] = rb[hi] - 128.0

    gidx_all, rlo_all, rhi_all = [], [], []
    for c in range(NCORES):
        gidx_all.append(np.ascontiguousarray(
            np.tile(vals[c].reshape(-1, 16).T, (8, 1))))
        rlo_all.append(np.ascontiguousarray(
            rel_by_col[c].T).astype(NPBF16))
        if NX:
            rhi_all.append(np.ascontiguousarray(
                rhi[c].T).astype(NPBF16))
        else:
            rhi_all.append(np.zeros((128, 1), NPBF16))

    # column schedule in emission order. Banks = runs of <=4
    # emission-consecutive pairs with contiguous global k (never
    # crossing a (core, half) boundary). One psum accumulation group
    # (2KB zero region) per bank.
    cols = []
    banks = []          # (k0, npairs)
    ei = 0
    while ei < NPAIR:
        k0 = emit_pairs[ei]
        npk = 1
        while (npk < 4 and ei + npk < NPAIR
               and emit_pairs[ei + npk] == k0 + npk
               and (k0 % KOWN) // KA == ((k0 + npk) % KOWN) // KA
               and (k0 + npk) // KOWN == k0 // KOWN):
            npk += 1
        banks.append((int(k0), npk))
        ei += npk
    bank_of_k = {}
    for bi, (k0, npk) in enumerate(banks):
        for s in range(npk):
            bank_of_k[k0 + s] = (bi, s)
    lastA_col = -1
    for ei, k in enumerate(emit_pairs):
        c0 = int(col_base_of_k[k])
        c1 = c0 + int(ncols_k[k])
        bi, slot = bank_of_k[k]
        k0, npk = banks[bi]
        for c in range(c0, c1):
            last = c == c1 - 1
            xid = int(xid_of[c]) if crossing[c] else -1
            bank_last_pair = slot == npk - 1
            cols.append(dict(
                k=int(k), first=(c == c0), last=last, xid=xid,
                bank=bi, slot=slot,
                bank_start=(c == c0 and slot == 0),
                stop_even=(last and xid < 0 and bank_last_pair),
                stop_odd=(last and xid >= 0 and bank_last_pair),
                spill=(last and bank_last_pair),
            ))
        if not half_of_k[k]:
            lastA_col = len(cols) - 1

    meta = dict(NSH=NSH, NTOWN=NTOWN, NT=NT, NPAIR=NPAIR, NCOL=NCOL,
                NPOS=NPOS, NX=max(NX, 1), cols=cols, banks=banks,
                KOWN=KOWN, KA=KA, lastA_col=lastA_col, n_nodes=n_nodes)
    return meta, gidx_all, rlo_all, rhi_all, deg_tiles


def build(meta):
    NSH, NTOWN, NT = meta["NSH"], meta["NTOWN"], meta["NT"]
    NCOL, NPOS, NX = meta["NCOL"], meta["NPOS"], meta["NX"]
    NPAIR, KOWN, KA = meta["NPAIR"], meta["KOWN"], meta["KA"]
    cols, banks = meta["cols"], meta["banks"]
    lastA_col = meta["lastA_col"]
    GA = 2 * KA          # own tiles in half A
    NGRP = (NTOWN + 7) // 8
    GRPA = GA // 8       # full 8-tile post groups entirely inside half A

    nc = bacc.Bacc("TRN2", target_bir_lowering=False, debug=False,
                   num_devices=NCORES, num_swdge_queues=4)

    xse = nc.dram_tensor("xs", [128, NTOWN * D], F32, kind="ExternalInput")
    dege = nc.dram_tensor("deg", [128, NTOWN], F32, kind="ExternalInput")
    gidxe = nc.dram_tensor("gidx", [128, NPOS // 16], I16,
                           kind="ExternalInput")
    rloe = nc.dram_tensor("rlo", [128, NCOL], BF16, kind="ExternalInput")
    rhie = nc.dram_tensor("rhi", [128, NX], BF16, kind="ExternalInput")
    W1e = nc.dram_tensor("W1", [D, D], BF16, kind="ExternalInput")
    W2e = nc.dram_tensor("W2", [D, D], BF16, kind="ExternalInput")
    b1e = nc.dram_tensor("b1bc", [128, D], F32, kind="ExternalInput")
    b2e = nc.dram_tensor("b2bc", [128, D], F32, kind="ExternalInput")
    woute = nc.dram_tensor("woutbc", [128, D], F32, kind="ExternalInput")
    boute = nc.dram_tensor("boutbc", [128, 1], F32, kind="ExternalInput")
    idente = nc.dram_tensor("identbf", [128, 128], BF16,
                            kind="ExternalInput")
    iotaxe = nc.dram_tensor("iotax", [128, 128 * CB], BF16,
                            kind="ExternalInput")
    oute = nc.dram_tensor("out", [128, NTOWN], F32, kind="ExternalOutput")

    T0 = nc.dram_tensor("T0", [NSH, 128], BF16)
    T1 = nc.dram_tensor("T1", [NSH, 128], BF16)
    # pair-major accumulators, one per dst half:
    # accA row = (dstcore*KA + kl)*256 + p*2 + e
    KB = KOWN - KA
    accH = [nc.dram_tensor("accA", [NCORES * KA * 256, D], BF16),
            nc.dram_tensor("accB", [NCORES * KB * 256, D], BF16)]
    shardH = [[nc.dram_tensor(f"shard{L}{h}",
                              [(KA if h == 0 else KB) * 256, D], BF16)
               for h in (0, 1)] for L in (0, 1)]

    def pairmaj(apx, e=2):
        # DRAM [(k p e), d] viewed as [128, k, e, d]
        return apx.rearrange("(k p e) d -> p k e d", p=128, e=e)

    with tile.TileContext(nc) as tc:
        with (
            tc.tile_pool(name="const", bufs=1) as pool,
            tc.tile_pool(name="msg", bufs=6) as msgpool,
            tc.tile_pool(name="slo", bufs=3) as slopool,
            tc.tile_pool(name="shi", bufs=3) as shipool,
            tc.tile_pool(name="stage", bufs=4) as stagepool,
            tc.tile_pool(name="aggT", bufs=2) as aggTpool,
            tc.tile_pool(name="hg", bufs=2) as hgpool,
            tc.tile_pool(name="aggps", bufs=4, space="PSUM") as aggps,
            tc.tile_pool(name="trps", bufs=2, space="PSUM") as trps,
            tc.tile_pool(name="mmps", bufs=2, space="PSUM") as mmps,
        ):
            # ---- constants ----
            gidx_t = pool.tile([128, NPOS // 16], I16, tag="gidx")
            rlo_t = pool.tile([128, NCOL], BF16, tag="rlo")
            rhi_t = pool.tile([128, NX], BF16, tag="rhi")
            nc.scalar.dma_start(out=gidx_t[:], in_=gidxe[:])
            nc.scalar.dma_start(out=rlo_t[:], in_=rloe[:])
            nc.scalar.dma_start(out=rhi_t[:], in_=rhie[:])
            W1_t = pool.tile([D, D], BF16, tag="w1")
            W2_t = pool.tile([D, D], BF16, tag="w2")
            b1_t = pool.tile([128, D], F32, tag="b1")
            b2_t = pool.tile([128, D], F32, tag="b2")
            wout_t = pool.tile([128, D], F32, tag="wout")
            bout_t = pool.tile([128, 1], F32, tag="bout")
            ident_t = pool.tile([128, 128], BF16, tag="ident")
            nc.scalar.dma_start(out=W1_t[:], in_=W1e[:])
            nc.scalar.dma_start(out=W2_t[:], in_=W2e[:])
            nc.scalar.dma_start(out=b1_t[:], in_=b1e[:])
            nc.scalar.dma_start(out=b2_t[:], in_=b2e[:])
            nc.scalar.dma_start(out=wout_t[:], in_=woute[:])
            nc.scalar.dma_start(out=bout_t[:], in_=boute[:])
            nc.scalar.dma_start(out=ident_t[:], in_=idente[:])
            deg_t = pool.tile([128, NTOWN], F32, tag="deg")
            nc.sync.dma_start(out=deg_t[:], in_=dege[:])
            dinv_t = pool.tile([128, NTOWN], F32, tag="dinv")
            nc.scalar.activation(dinv_t[:], deg_t[:],
                                 mybir.ActivationFunctionType.Sqrt)
            nc.vector.reciprocal(dinv_t[:], dinv_t[:])
            # iota_x[p, j, c] = j (all inner strides 1 for 2x is_equal)
            iota_x = pool.tile([128, 128, CB], BF16, tag="iotax")
            nc.scalar.dma_start(
                out=iota_x[:],
                in_=iotaxe.ap().rearrange("p (j c) -> p j c", c=CB))

            xs_t = pool.tile([128, NTOWN, D], F32, tag="xs")
            nc.sync.dma_start(
                out=xs_t[:],
                in_=xse.ap().rearrange("p (g d) -> p g d", d=D))

            # ---- u0 = dinv * x into junk-row staging ----
            tjunk = pool.tile([128, NTOWN, 128], BF16, tag="tjunk")
            nc.vector.memset(tjunk[:], 0.0)
            dvb = dinv_t[:].unsqueeze(2).broadcast_to([128, NTOWN, D])
            nc.vector.tensor_tensor(tjunk[:, :, 0:D], xs_t[:], dvb,
                                    mybir.AluOpType.mult)
            nc.sync.dma_start(
                out=T0.ap().rearrange("(g p) d -> p g d", p=128),
                in_=tjunk[:])

            NGB = (NCOL + GB - 1) // GB

            def rs_half(L, h):
                nc.gpsimd.collective_compute(
                    "ReduceScatter", mybir.AluOpType.add,
                    replica_groups=[list(range(NCORES))],
                    ins=[accH[h].ap().opt()],
                    outs=[shardH[L][h].ap().opt()])

            def agg_layer(L):
                Tbl = T0 if L == 0 else T1
                psum_live = {}
                sls, shs, x0s = {}, {}, {}
                for gb in range(NGB):
                    g0 = gb * GB
                    gbc = min(GB, NCOL - g0)
                    mt = msgpool.tile([128, GB, 128], BF16, tag="m")
                    nc.gpsimd.dma_gather(
                        mt[:, :gbc, :], Tbl[:],
                        gidx_t[:, g0 * 8:(g0 + gbc) * 8],
                        num_idxs=gbc * 128, num_idxs_reg=gbc * 128,
                        elem_size=128, single_packet=False,
                        queue_num=gb % 4)
                    for sb in range(g0 // CB,
                                    (g0 + gbc + CB - 1) // CB):
                        c0 = sb * CB
                        bc = min(CB, NCOL - c0)
                        sl = slopool.tile([128, 128, CB], BF16, tag="sl")
                        nc.vector.tensor_tensor(
                            sl[:, :, :bc], iota_x[:, :, :bc],
                            rlo_t[:, c0:c0 + bc].unsqueeze(1).broadcast_to(
                                [128, 128, bc]),
                            mybir.AluOpType.is_equal)
                        sls[sb] = sl
                        xids = [cols[c]["xid"] for c in range(c0, c0 + bc)
                                if cols[c]["xid"] >= 0]
                        if xids:
                            x0, nxb = xids[0], len(xids)
                            assert xids == list(range(x0, x0 + nxb))
                            x0s[sb] = x0
                            sh = shipool.tile([128, 128, CB], BF16,
                                              tag="sh")
                            nc.vector.tensor_tensor(
                                sh[:, :, :nxb], iota_x[:, :, :nxb],
                                rhi_t[:, x0:x0 + nxb].unsqueeze(1)
                                .broadcast_to([128, 128, nxb]),
                                mybir.AluOpType.is_equal)
                            shs[sb] = sh
                    for c in range(g0, g0 + gbc):
                        info = cols[c]
                        bi = info["bank"]
                        slot = info["slot"]
                        if bi not in psum_live:
                            psum_live[bi] = aggps.tile(
                                [128, 8, 64], F32, tag="agg", name="aggb")
                        pt = psum_live[bi]
                        j = c - g0
                        sb = c // CB
                        nc.tensor.matmul(
                            pt[:, 2 * slot, :],
                            sls[sb][:, :, c - sb * CB], mt[:, j, 0:D],
                            start=info["bank_start"],
                            stop=info["stop_even"])
                        if info["xid"] >= 0:
                            xj = info["xid"] - x0s[sb]
                            nc.tensor.matmul(
                                pt[:, 2 * slot + 1, :],
                                shs[sb][:, :, xj], mt[:, j, 0:D],
                                start=False, stop=info["stop_odd"])
                        if info["spill"]:
                            k0, npk = banks[bi]
                            dc, kl = k0 // KOWN, k0 % KOWN
                            if kl < KA:
                                av, r0 = accH[0], dc * KA + kl
                            else:
                                av, r0 = accH[1], dc * (KOWN - KA) + (
                                    kl - KA)
                            st = stagepool.tile([128, 8, 64], BF16,
                                                tag="st")
                            nc.vector.tensor_copy(st[:, :2 * npk, :],
                                                  pt[:, :2 * npk, :])
                            nc.sync.dma_start(
                                out=pairmaj(
                                    av[r0 * 256:(r0 + npk) * 256, :]),
                                in_=st[:, :2 * npk, :].rearrange(
                                    "p (k e) d -> p k e d", e=2))
                            del psum_live[bi]
                # Collectives are issued from the (in-order) Pool queue;
                # emitting them only after every gather of the layer has
                # been issued keeps the queue from stalling gather issue
                # while the CC cores run. Half A's RS still overlaps the
                # half-B drain/matmul tail on the other engines.
                rs_half(L, 0)
                post_half(L, 0)
                rs_half(L, 1)
                post_half(L, 1)
                assert not psum_live, psum_live.keys()

            o_t = pool.tile([128, NTOWN], F32, tag="o")
            sh_ts = {}

            def post_half(L, h):
                """RS_h result -> agg_u -> @W -> epilogue for that half."""
                if h == 0:
                    sh_t = pool.tile([128, NTOWN, D], BF16, tag="shards",
                                     name="sh_t")
                    sh_ts[L] = sh_t
                    ga, gb_ = 0, 2 * KA
                    grps = range(0, GRPA)
                else:
                    sh_t = sh_ts[L]
                    ga, gb_ = 2 * KA, NTOWN
                    grps = range(GRPA, NGRP)
                nc.sync.dma_start(
                    out=sh_t[:, ga:gb_, :].rearrange(
                        "p (k e) d -> p k e d", e=2),
                    in_=pairmaj(shardH[L][h].ap()))
                nc.vector.tensor_tensor(sh_t[:, ga:gb_, :],
                                        sh_t[:, ga:gb_, :],
                                        tjunk[:, ga:gb_, 0:D],
                                        mybir.AluOpType.add)
                W_t = W1_t if L == 0 else W2_t
                b_t = b1_t if L == 0 else b2_t
                for tg8 in grps:
                    tg = tg8 * 8
                    ng = min(8, NTOWN - tg)
                    tp = trps.tile([64, 8, 128], BF16, tag="tr")
                    for t in range(tg, tg + ng):
                        nc.tensor.transpose(tp[:, t - tg, :],
                                            sh_t[:, t, :], ident_t[:])
                    aT = aggTpool.tile([64, 8, 128], BF16, tag="aT")
                    nc.vector.tensor_copy(aT[:, :ng, :], tp[:, :ng, :])
                    mp = mmps.tile([128, 8, 64], F32, tag="mm")
                    for t in range(tg, tg + ng):
                        nc.tensor.matmul(mp[:, t - tg, :],
                                         aT[:, t - tg, :], W_t[:])
                    hg = hgpool.tile([128, 8, 64], F32, tag="hg")
                    dvg = dinv_t[:, tg:tg + ng].unsqueeze(2).broadcast_to(
                        [128, ng, 64])
                    bbg = b_t[:].unsqueeze(1).broadcast_to([128, ng, 64])
                    nc.vector.tensor_tensor(hg[:, :ng, :], mp[:, :ng, :],
                                            dvg, mybir.AluOpType.mult)
                    nc.vector.tensor_tensor(hg[:, :ng, :], hg[:, :ng, :],
                                            bbg, mybir.AluOpType.add)
                    nc.scalar.activation(hg[:, :ng, :], hg[:, :ng, :],
                                         mybir.ActivationFunctionType.Relu)
                    if L == 0:
                        nc.vector.tensor_tensor(
                            tjunk[:, tg:tg + ng, 0:D], hg[:, :ng, :],
                            dvg, mybir.AluOpType.mult)
                    else:
                        wbg = wout_t[:].unsqueeze(1).broadcast_to(
                            [128, ng, 64])
                        nc.vector.tensor_tensor(hg[:, :ng, :],
                                                hg[:, :ng, :], wbg,
                                                mybir.AluOpType.mult)
                        nc.vector.tensor_reduce(
                            o_t[:, tg:tg + ng], hg[:, :ng, :],
                            axis=mybir.AxisListType.X,
                            op=mybir.AluOpType.add)
                if h == 1:
                    if L == 0:
                        nc.sync.dma_start(
                            out=T1.ap().rearrange("(g p) d -> p g d",
                                                  p=128),
                            in_=tjunk[:])
                    else:
                        nc.vector.tensor_scalar_add(o_t[:], o_t[:],
                                                    bout_t[:])
                        nc.sync.dma_start(out=oute[:], in_=o_t[:])

            for L in (0, 1):
                agg_layer(L)

    nc.compile()
    return nc


_CACHE = {}


def kernel(x, edge_index, batch, W1, b1, W2, b2, Wout, bout, _trace=False):
    x = np.asarray(x, np.float32)
    edge_index = np.asarray(edge_index)
    W1 = np.asarray(W1, np.float32)
    W2 = np.asarray(W2, np.float32)
    b1 = np.asarray(b1, np.float32)
    b2 = np.asarray(b2, np.float32)
    Wout = np.asarray(Wout, np.float32)
    bout = np.asarray(bout, np.float32).reshape(-1)
    N = x.shape[0]

    key = (N, edge_index.shape[1])
    if key not in _CACHE:
        meta, gidx_all, rlo_all, rhi_all, deg_tiles = preprocess(
            edge_index, N)
        nc = build(meta)
        _CACHE[key] = (meta, gidx_all, rlo_all, rhi_all, deg_tiles, nc)
    meta, gidx_all, rlo_all, rhi_all, deg_tiles, nc = _CACHE[key]
    NSH, NTOWN = meta["NSH"], meta["NTOWN"]

    identbf = np.eye(128, dtype=np.float32).astype(NPBF16)
    iotax = np.ascontiguousarray(np.broadcast_to(
        np.arange(128, dtype=np.float32)[None, :, None],
        (128, 128, CB)).reshape(128, 128 * CB)).astype(NPBF16)
    b1bc = np.tile(b1[None, :], (128, 1)).astype(np.float32)
    b2bc = np.tile(b2[None, :], (128, 1)).astype(np.float32)
    woutbc = np.tile(Wout.reshape(1, -1), (128, 1)).astype(np.float32)
    boutbc = np.full((128, 1), float(bout[0]), np.float32)
    W1bf = W1.astype(NPBF16)
    W2bf = W2.astype(NPBF16)

    in_maps = []
    for c in range(NCORES):
        xsh = np.zeros((NSH, D), np.float32)
        lo, hi = c * NSH, min((c + 1) * NSH, N)
        if hi > lo:
            xsh[:hi - lo] = x[lo:hi]
        xs = np.ascontiguousarray(
            xsh.reshape(NTOWN, 128, D).transpose(1, 0, 2).reshape(
                128, NTOWN * D))
        in_maps.append({
            "xs": xs, "deg": deg_tiles[c],
            "gidx": gidx_all[c], "rlo": rlo_all[c], "rhi": rhi_all[c],
            "W1": W1bf, "W2": W2bf, "b1bc": b1bc, "b2bc": b2bc,
            "woutbc": woutbc, "boutbc": boutbc, "identbf": identbf,
            "iotax": iotax,
        })

    res = bass_utils.run_bass_kernel_spmd(
        nc, in_maps, core_ids=list(range(NCORES)), trace=_trace)

    out = np.zeros(N, np.float32)
    for c in range(NCORES):
        o = res.results[c]["out"]  # [128, NTOWN]
        arr = o.T.ravel()          # node-major: g*128 + p
        lo, hi = c * NSH, min((c + 1) * NSH, N)
        if hi > lo:
            out[lo:hi] = arr[:hi - lo]
    if _trace:
        return out, res.exec_time_ns
    return out
